# revision 17
# baseline (speedup 1.0000x reference)
"""BatchBlur: depthwise 15x15 conv with per-sample kernels, reflection pad 7.

x: (32, 3, 512, 512) f32, kernel: (32, 15, 15) f32 -> out (32, 3, 512, 512) f32.

Strategy: pure data parallel over batch, 4 samples per core on 8 cores.
Host: reflection-pad x to (., 526, 526), cast to fp16, and build banded
"vertical" matrices A[s, k, dx, m] = kern[s, k-m, dx] (0 <= k-m < 15).
Device: per 64-row strip, accumulate 15 matmuls in PSUM (one per horizontal
tap dx):
  out[m, n] += sum_k A_dx[k, m] * xp[r0+k, n+dx]
which realizes the full 2D conv (vertical taps inside the band matrix,
horizontal taps via rhs free-dim offsets). Two images are processed
concurrently in the two 64-column halves of the PE array via
tile_position=(0,0)/(0,64), so every streamed column produces 128 output
rows. fp16 operands keep the PE at 1 cycle/row (fp32 accumulation in PSUM;
~3e-4 relative error) and halve the input DMA bytes. 512 = 8*64 exactly,
so there is no partial tail strip.
"""
import os
import sys

for _p in ("/opt/trn_rl_repo", "/root/.axon_site/_ro/trn_rl_repo"):
    if _p not in sys.path and os.path.isdir(_p):
        sys.path.insert(0, _p)

import numpy as np

import concourse.mybir as mybir
import concourse.tile as tile
from concourse import bacc
from concourse.bass_utils import run_bass_kernel_spmd

L = 15           # blur kernel size
P = L // 2       # reflection pad
B, C, H, W = 32, 3, 512, 512
N_CORES = 8
BS = B // N_CORES            # samples per core
NIMG = BS * C                # channel images per core
HP, WP = H + 2 * P, W + 2 * P  # 526
M_STRIP = 64                 # output rows per strip (col-group size)
K_STRIP = M_STRIP + L - 1    # 78 input rows per strip
N_STRIPS = H // M_STRIP      # 8, exact
N_WARMUP = 44                # dummy matmuls to release the HAM clock gate

F16 = mybir.dt.float16
F32 = mybir.dt.float32

_program_cache = None


def _build_program():
    nc = bacc.Bacc("TRN2", target_bir_lowering=False, debug=False)
    xp_d = nc.dram_tensor("xp", [NIMG, HP, WP], F16, kind="ExternalInput").ap()
    a_d = nc.dram_tensor("a", [BS, K_STRIP, L, M_STRIP], F16,
                         kind="ExternalInput").ap()
    out_d = nc.dram_tensor("out", [NIMG, H, W], F32, kind="ExternalOutput").ap()

    with tile.TileContext(nc) as tc:
        with (
            tc.tile_pool(name="aconst", bufs=1) as apool,
            tc.tile_pool(name="warm", bufs=1) as wpool,
            tc.tile_pool(name="xin", bufs=6) as xpool,
            tc.tile_pool(name="oout", bufs=4) as opool,
            tc.tile_pool(name="psum", bufs=4, space="PSUM") as psum,
            tc.tile_pool(name="psumw", bufs=1, space="PSUM") as psumw,
        ):
            # HAM warm-up: a burst of tiny matmuls on a zeroed scratch tile
            # keeps the PE busy while the first input DMAs are in flight, so
            # the real matmul stream starts at 2.4 GHz instead of 1.2.
            wsrc = wpool.tile([128, 64], mybir.dt.bfloat16)
            nc.gpsimd.memset(wsrc[:], 0.0)
            wacc = psumw.tile([64, 64], F32)
            for _ in range(N_WARMUP):
                nc.tensor.matmul(wacc[:], wsrc[:, :64], wsrc[:], start=True,
                                 stop=True)

            # first strip pair's image rows: issued before the A load so the
            # DMA queue delivers the first matmuls' dependencies earliest
            xp_first = []
            for img in range(2):
                t = xpool.tile([K_STRIP, WP], F16, tag="xp_t",
                               name=f"xpf{img}")
                nc.sync.dma_start(out=t[:], in_=xp_d[img, 0:K_STRIP, :])
                xp_first.append(t)

            # per-sample band-matrix tiles: separate tiles => separate
            # dependency tracking. Sample 0 loads now; later samples load
            # lazily (one pair ahead of first use) so those transfers don't
            # delay the early image-row DMAs on the input queue.
            a_t = [
                apool.tile([K_STRIP, L, M_STRIP], F16, tag=f"a{s}",
                           name=f"a{s}")
                for s in range(BS)
            ]
            nc.sync.dma_start(out=a_t[0][:], in_=a_d[0])

            a_loaded = 0  # highest sample index whose band matrices are queued
            for pair in range(NIMG // 2):
                img_a, img_b = 2 * pair, 2 * pair + 1
                smp_a, smp_b = img_a // C, img_b // C
                # prefetch band matrices needed by the NEXT pair
                for s_need in ((2 * pair + 2) // C, (2 * pair + 3) // C):
                    if s_need < BS and s_need > a_loaded:
                        nc.sync.dma_start(out=a_t[s_need][:], in_=a_d[s_need])
                        a_loaded = s_need

                for s in range(N_STRIPS):
                    r0 = s * M_STRIP
                    if pair == 0 and s == 0:
                        xa, xb = xp_first
                    else:
                        xa = xpool.tile([K_STRIP, WP], F16, tag="xp_t",
                                        name="xa")
                        nc.sync.dma_start(
                            out=xa[:], in_=xp_d[img_a, r0:r0 + K_STRIP, :])
                        xb = xpool.tile([K_STRIP, WP], F16, tag="xp_t",
                                        name="xb")
                        nc.sync.dma_start(
                            out=xb[:], in_=xp_d[img_b, r0:r0 + K_STRIP, :])
                    acc = psum.tile([128, W], F32)
                    for dx in range(L):
                        nc.tensor.matmul(
                            acc[0:M_STRIP],
                            a_t[smp_a][:, dx, :],
                            xa[:, dx:dx + W],
                            start=(dx == 0),
                            stop=(dx == L - 1),
                            tile_position=(0, 0),
                        )
                        nc.tensor.matmul(
                            acc[M_STRIP:2 * M_STRIP],
                            a_t[smp_b][:, dx, :],
                            xb[:, dx:dx + W],
                            start=(dx == 0),
                            stop=(dx == L - 1),
                            tile_position=(0, M_STRIP),
                        )
                    o_t = opool.tile([128, W], F32)
                    nc.vector.tensor_copy(out=o_t[:], in_=acc[:])
                    # stores issue from the (otherwise idle) Scalar queue so
                    # they never block the input-DMA stream on the Sync queue
                    nc.scalar.dma_start(
                        out=out_d[img_a, r0:r0 + M_STRIP, :],
                        in_=o_t[0:M_STRIP])
                    nc.scalar.dma_start(
                        out=out_d[img_b, r0:r0 + M_STRIP, :],
                        in_=o_t[M_STRIP:2 * M_STRIP])
    nc.compile()
    return nc


def prepare_in_maps(x: np.ndarray, kern: np.ndarray) -> list:
    # host-side reflection pad, cast to fp16 for half the DMA bytes
    xp = np.pad(x, ((0, 0), (0, 0), (P, P), (P, P)), mode="reflect")
    xp = np.ascontiguousarray(
        xp.reshape(B * C, HP, WP).astype(np.float16))

    # band matrices: a_all[s, k, dx, m] = kern[s, k-m, dx] for 0 <= k-m < L
    a_all = np.zeros((B, K_STRIP, L, M_STRIP), dtype=np.float16)
    m_idx = np.arange(M_STRIP)
    for dy in range(L):
        a_all[:, m_idx + dy, :, m_idx] = kern[:, dy, :].astype(np.float16)

    return [
        {
            "xp": xp[c * NIMG:(c + 1) * NIMG],
            "a": a_all[c * BS:(c + 1) * BS],
        }
        for c in range(N_CORES)
    ]


def kernel(x: np.ndarray, kernel: np.ndarray) -> np.ndarray:
    global _program_cache
    x = np.asarray(x, dtype=np.float32)
    kern = np.asarray(kernel, dtype=np.float32)

    in_maps = prepare_in_maps(x, kern)
    if _program_cache is None:
        _program_cache = _build_program()
    nc = _program_cache

    res = run_bass_kernel_spmd(nc, in_maps, core_ids=list(range(N_CORES)))
    out = np.concatenate([r["out"] for r in res.results], axis=0)
    return out.reshape(B, C, H, W)


# revision 18
# speedup vs baseline: 1.8738x; 1.8738x over previous
"""BatchBlur: depthwise 15x15 conv with per-sample kernels, reflection pad 7.

x: (32, 3, 512, 512) f32, kernel: (32, 15, 15) f32 -> out (32, 3, 512, 512) f32.

Strategy: pure data parallel over batch, 4 samples per core on 8 cores.
Host: reflection-pad x to (., 526, 526), cast to fp16, and build banded
"vertical" matrices A[s, k, dx, m] = kern[s, k-m, dx] (0 <= k-m < 15).
Device: per 64-row strip, accumulate 15 matmuls in PSUM (one per horizontal
tap dx):
  out[m, n] += sum_k A_dx[k, m] * xp[r0+k, n+dx]
which realizes the full 2D conv (vertical taps inside the band matrix,
horizontal taps via rhs free-dim offsets). Two images are processed
concurrently in the two 64-column halves of the PE array via
tile_position=(0,0)/(0,64), so every streamed column produces 128 output
rows. fp16 operands keep the PE at 1 cycle/row (fp32 accumulation in PSUM;
~3e-4 relative error) and halve the input DMA bytes. 512 = 8*64 exactly,
so there is no partial tail strip.
"""
import os
import sys

for _p in ("/opt/trn_rl_repo", "/root/.axon_site/_ro/trn_rl_repo"):
    if _p not in sys.path and os.path.isdir(_p):
        sys.path.insert(0, _p)

import numpy as np

import concourse.mybir as mybir
import concourse.tile as tile
from concourse import bacc
from concourse.bass_utils import run_bass_kernel_spmd

L = 15           # blur kernel size
P = L // 2       # reflection pad
B, C, H, W = 32, 3, 512, 512
N_CORES = 8
BS = B // N_CORES            # samples per core
NIMG = BS * C                # channel images per core
HP, WP = H + 2 * P, W + 2 * P  # 526
M_STRIP = 64                 # output rows per strip (col-group size)
K_STRIP = M_STRIP + L - 1    # 78 input rows per strip
N_STRIPS = H // M_STRIP      # 8, exact
N_WARMUP = 80                # dummy matmuls to release the HAM clock gate

F16 = mybir.dt.float16
F32 = mybir.dt.float32

_program_cache = None


def _build_program():
    nc = bacc.Bacc("TRN2", target_bir_lowering=False, debug=False)
    xp_d = nc.dram_tensor("xp", [NIMG, HP, WP], F16, kind="ExternalInput").ap()
    a_d = nc.dram_tensor("a", [BS, K_STRIP, L, M_STRIP], F16,
                         kind="ExternalInput").ap()
    out_d = nc.dram_tensor("out", [NIMG, H, W], F32, kind="ExternalOutput").ap()

    with tile.TileContext(nc) as tc:
        with (
            tc.tile_pool(name="aconst", bufs=1) as apool,
            tc.tile_pool(name="warm", bufs=1) as wpool,
            tc.tile_pool(name="xin", bufs=6) as xpool,
            tc.tile_pool(name="oout", bufs=4) as opool,
            tc.tile_pool(name="psum", bufs=4, space="PSUM") as psum,
            tc.tile_pool(name="psumw", bufs=1, space="PSUM") as psumw,
        ):
            # HAM warm-up: a burst of tiny matmuls on a zeroed scratch tile
            # keeps the PE busy while the first input DMAs are in flight, so
            # the real matmul stream starts at 2.4 GHz instead of 1.2.
            wsrc = wpool.tile([128, 64], mybir.dt.bfloat16)
            nc.gpsimd.memset(wsrc[:], 0.0)
            wacc = psumw.tile([64, 64], F32)
            for _ in range(N_WARMUP):
                nc.tensor.matmul(wacc[:], wsrc[:, :64], wsrc[:], start=True,
                                 stop=True)

            # first strip pair's image rows: issued before the A load so the
            # DMA queue delivers the first matmuls' dependencies earliest
            xp_first = []
            for img in range(2):
                t = xpool.tile([K_STRIP, WP], F16, tag="xp_t",
                               name=f"xpf{img}")
                nc.sync.dma_start(out=t[:], in_=xp_d[img, 0:K_STRIP, :])
                xp_first.append(t)

            # per-sample band-matrix tiles: separate tiles => separate
            # dependency tracking. Sample 0 loads now; later samples load
            # lazily (one pair ahead of first use) so those transfers don't
            # delay the early image-row DMAs on the input queue.
            a_t = [
                apool.tile([K_STRIP, L, M_STRIP], F16, tag=f"a{s}",
                           name=f"a{s}")
                for s in range(BS)
            ]
            nc.sync.dma_start(out=a_t[0][:], in_=a_d[0])

            a_loaded = 0  # highest sample index whose band matrices are queued
            for pair in range(NIMG // 2):
                img_a, img_b = 2 * pair, 2 * pair + 1
                smp_a, smp_b = img_a // C, img_b // C
                # prefetch band matrices needed by the NEXT pair
                for s_need in ((2 * pair + 2) // C, (2 * pair + 3) // C):
                    if s_need < BS and s_need > a_loaded:
                        nc.sync.dma_start(out=a_t[s_need][:], in_=a_d[s_need])
                        a_loaded = s_need

                for s in range(N_STRIPS):
                    r0 = s * M_STRIP
                    if pair == 0 and s == 0:
                        xa, xb = xp_first
                    else:
                        xa = xpool.tile([K_STRIP, WP], F16, tag="xp_t",
                                        name="xa")
                        nc.sync.dma_start(
                            out=xa[:], in_=xp_d[img_a, r0:r0 + K_STRIP, :])
                        xb = xpool.tile([K_STRIP, WP], F16, tag="xp_t",
                                        name="xb")
                        nc.sync.dma_start(
                            out=xb[:], in_=xp_d[img_b, r0:r0 + K_STRIP, :])
                    acc = psum.tile([128, W], F32)
                    for dx in range(L):
                        nc.tensor.matmul(
                            acc[0:M_STRIP],
                            a_t[smp_a][:, dx, :],
                            xa[:, dx:dx + W],
                            start=(dx == 0),
                            stop=(dx == L - 1),
                            tile_position=(0, 0),
                        )
                        nc.tensor.matmul(
                            acc[M_STRIP:2 * M_STRIP],
                            a_t[smp_b][:, dx, :],
                            xb[:, dx:dx + W],
                            start=(dx == 0),
                            stop=(dx == L - 1),
                            tile_position=(0, M_STRIP),
                        )
                    o_t = opool.tile([128, W], F32)
                    nc.vector.tensor_copy(out=o_t[:], in_=acc[:])
                    # stores issue from the (otherwise idle) Scalar queue so
                    # they never block the input-DMA stream on the Sync queue
                    nc.scalar.dma_start(
                        out=out_d[img_a, r0:r0 + M_STRIP, :],
                        in_=o_t[0:M_STRIP])
                    nc.scalar.dma_start(
                        out=out_d[img_b, r0:r0 + M_STRIP, :],
                        in_=o_t[M_STRIP:2 * M_STRIP])
    nc.compile()
    return nc


def prepare_in_maps(x: np.ndarray, kern: np.ndarray) -> list:
    # host-side reflection pad, cast to fp16 for half the DMA bytes
    xp = np.pad(x, ((0, 0), (0, 0), (P, P), (P, P)), mode="reflect")
    xp = np.ascontiguousarray(
        xp.reshape(B * C, HP, WP).astype(np.float16))

    # band matrices: a_all[s, k, dx, m] = kern[s, k-m, dx] for 0 <= k-m < L
    a_all = np.zeros((B, K_STRIP, L, M_STRIP), dtype=np.float16)
    m_idx = np.arange(M_STRIP)
    for dy in range(L):
        a_all[:, m_idx + dy, :, m_idx] = kern[:, dy, :].astype(np.float16)

    return [
        {
            "xp": xp[c * NIMG:(c + 1) * NIMG],
            "a": a_all[c * BS:(c + 1) * BS],
        }
        for c in range(N_CORES)
    ]


def kernel(x: np.ndarray, kernel: np.ndarray) -> np.ndarray:
    global _program_cache
    x = np.asarray(x, dtype=np.float32)
    kern = np.asarray(kernel, dtype=np.float32)

    in_maps = prepare_in_maps(x, kern)
    if _program_cache is None:
        _program_cache = _build_program()
    nc = _program_cache

    res = run_bass_kernel_spmd(nc, in_maps, core_ids=list(range(N_CORES)))
    out = np.concatenate([r["out"] for r in res.results], axis=0)
    return out.reshape(B, C, H, W)
